# revision 21
# baseline (speedup 1.0000x reference)
"""Causal self-attention (B=4, T=2048, C=2048, H=16, rope) on 8 trn2 cores.

Sharding: tensor-parallel over heads — 2 heads per core. Each core computes
q/k/v projections for its head slice from the full x, runs causal attention,
and produces a partial output projection y_c = attn_c @ wo[:, slice].T.
The host sums the 8 partial y tensors (row-parallel linear unshard).

v2 layout (fp16 matmuls; "T" suffix = contraction dim on SBUF partitions):
  qT/kT [d=128, t]  <- wT (stationary) x xT (moving) matmuls + rope on DVE
  v     [t=128, d]  <- xT-tile (stationary) x wvT (moving) matmuls, ACT copy
  S^T   [j, 2*i]    <- head-paired [128,1024] PSUM tile; causal mask written
                       by an identity matmul (start=True), S accumulates onto
                       it (start=False) with diagonal tiles column-trimmed
  P^T = exp(S^T * scale) on ACT (one op per jt for both heads); row-sum
                       accumulated on DVE (fp16); one ones-matmul per head
                       broadcasts it; reciprocal_approx_fast + multiply on DVE
  attn^T [d, i]     <- v-tile (stationary) x P^T-half (moving), PSUM-accum
  y[n, j]           <- attnT-tile (stationary) x woT (moving); PSUM->SBUF
                       copies split between ACT and DVE

Scheduling: the attention jt-loop is software-pipelined (S/exp for jt issue
before AV for jt-1) so the exp latency hides under PE work, and phase C of
block a is emitted after phase A of block a+1 so the softmax tail latency
hides under the next block's projection matmuls.
"""

import numpy as np

import concourse.bass as bass
import concourse.mybir as mybir
import concourse.tile as tile
from concourse.vector_clock import ScopedClock
from concourse.bass_utils import run_bass_kernel_spmd

# ---------------------------------------------------------------- tile patch
# The pinned walrus codegen accepts at most ONE sync-wait per hardware
# instruction; Tile attaches several. Split extras onto same-engine NOPs.

_MAX_WAITS = 1
_orig_add_instruction = tile.TileContext._add_instruction


def _split_add_instruction(self, inst):
    si = getattr(inst, "sync_info", None)
    if si is not None and len(si.on_wait) > _MAX_WAITS:
        waits = list(si.on_wait)
        extras, keep = waits[:-_MAX_WAITS], waits[-_MAX_WAITS:]
        inst.sync_info = mybir.SyncInfo(on_wait=keep, on_update=list(si.on_update))
        for i in range(0, len(extras), _MAX_WAITS):
            nop = mybir.InstNoOp(
                name=f"{inst.name}-ws{i}",
                sync_info=mybir.SyncInfo(on_wait=extras[i : i + _MAX_WAITS], on_update=[]),
                engine=inst.engine,
                bass_nofuse=True,
            )
            _orig_add_instruction(self, nop)
    _orig_add_instruction(self, inst)


def _patched_drain_and_barrier(self, tick_clock, wait_clock):
    nc = self.nc
    drain_inst = nc.sync.drain()
    wait_clock.add_sem_waits(drain_inst.ins, ScopedClock({None: tick_clock.global_clock}))
    si = drain_inst.ins.sync_info
    if si is not None and len(si.on_wait) > 1:
        waits = list(si.on_wait)
        drain_inst.ins.sync_info = mybir.SyncInfo(on_wait=waits[:1], on_update=list(si.on_update))
        for w in waits[1:]:
            extra = nc.sync.drain()
            extra.ins.sync_info = mybir.SyncInfo(on_wait=[w], on_update=[])
    nc.all_engine_barrier()
    assert self.sems is not None
    popped = nc._tile_sem_poison_stack.pop()
    assert popped is self._sem_poison
    nc.clear_and_free_semaphores(list(self.sems.allocated().values()))
    nc.all_engine_barrier()


tile.TileContext._add_instruction = _split_add_instruction
tile.TileContext._drain_and_barrier = _patched_drain_and_barrier

# ---------------------------------------------------------------- constants

B, T, C, H, D = 4, 2048, 2048, 16, 128
N_CORES = 8
HPC = H // N_CORES        # heads per core = 2
M = HPC * D               # per-core projection width = 256
BT = B * T
KT = C // 128             # 16 k-subtiles
SCALE = 1.0 / float(np.sqrt(D))
NEG = -30000.0            # pre-scale additive mask value; exp(scale*(s+NEG)) == 0

F32 = mybir.dt.float32
F32R = mybir.dt.float32r
F16 = mybir.dt.float16
BF16 = mybir.dt.bfloat16

DT_MM = F16
ALU = mybir.AluOpType
AF = mybir.ActivationFunctionType

TRIM_DIAG = True          # trim diagonal S matmul moving to [o:512]


def build_kernel(dt_mm=DT_MM, nrep=1):
    nc = bass.Bass("TRN2", target_bir_lowering=False, debug=False)

    xT = nc.dram_tensor("xT", [BT // 512, 128, KT, 512], dt_mm, kind="ExternalInput").ap()
    wqT = nc.dram_tensor("wqT", [C, M], dt_mm, kind="ExternalInput").ap()
    wkT = nc.dram_tensor("wkT", [C, M], dt_mm, kind="ExternalInput").ap()
    wvT = nc.dram_tensor("wvT", [C, M], dt_mm, kind="ExternalInput").ap()
    woT = nc.dram_tensor("woT", [M, C], dt_mm, kind="ExternalInput").ap()
    cosT = nc.dram_tensor("cosT", [D, T], F32, kind="ExternalInput").ap()
    sinT = nc.dram_tensor("sinT", [D, T], F32, kind="ExternalInput").ap()
    maskA = nc.dram_tensor("maskA", [128, 896], dt_mm, kind="ExternalInput").ap()
    ones = nc.dram_tensor("ones", [128, 128], F32R, kind="ExternalInput").ap()
    ident = nc.dram_tensor("ident", [128, 128], dt_mm, kind="ExternalInput").ap()
    y = nc.dram_tensor("y", [BT // 128, C // 512, 128, 512], dt_mm, kind="ExternalOutput").ap()

    with tile.TileContext(nc) as tc:
        with (
            tc.tile_pool(name="const", bufs=1) as constp,
            tc.tile_pool(name="cs", bufs=2) as csp,
            tc.tile_pool(name="xpool", bufs=3) as xpool,
            tc.tile_pool(name="qpool", bufs=2) as qpool,
            tc.tile_pool(name="kvpool", bufs=2) as kvpool,
            tc.tile_pool(name="attnpool", bufs=2) as attnpool,
            tc.tile_pool(name="ptpool", bufs=3) as ptpool,
            tc.tile_pool(name="ptsump", bufs=2) as ptsump,
            tc.tile_pool(name="tmp", bufs=4) as tmpp,
            tc.tile_pool(name="recp", bufs=2) as recp,
            tc.tile_pool(name="ystg", bufs=10) as ystg,
            tc.tile_pool(name="ps_main", bufs=2, space="PSUM") as ps_main,
            tc.tile_pool(name="ps_misc", bufs=2, space="PSUM") as ps_misc,
            tc.tile_pool(name="ps_av", bufs=2, space="PSUM") as ps_av,
        ):
            # ---- resident constants; wq first so phase A can start early
            wq_sb = constp.tile([128, KT, M], dt_mm, tag="wq")
            wk_sb = constp.tile([128, KT, M], dt_mm, tag="wk")
            wv_sb = constp.tile([128, KT, M], dt_mm, tag="wv")
            wo_sb = constp.tile([128, HPC, C], dt_mm, tag="wo")
            mask_sb = constp.tile([128, 896], dt_mm, tag="mask")
            ones_sb = constp.tile([128, 128], F32R, tag="ones")
            ident_sb = constp.tile([128, 128], dt_mm, tag="ident")
            wqR = wqT.rearrange("(ko p) m -> p ko m", p=128)
            for wc in range(4):  # chunked so the first matmuls start early
                nc.gpsimd.dma_start(wq_sb[:, 4 * wc : 4 * wc + 4, :], wqR[:, 4 * wc : 4 * wc + 4, :])
            nc.gpsimd.dma_start(wk_sb[:], wkT.rearrange("(ko p) m -> p ko m", p=128))
            nc.gpsimd.dma_start(wv_sb[:], wvT.rearrange("(ko p) m -> p ko m", p=128))
            nc.scalar.dma_start(mask_sb[:], maskA[:])
            nc.scalar.dma_start(ones_sb[:], ones[:])
            nc.scalar.dma_start(ident_sb[:], ident[:])
            nc.gpsimd.dma_start(wo_sb[:], woT.rearrange("(mh p) j -> p mh j", p=128))

            wqk = [wq_sb, wq_sb, wk_sb, wk_sb]

            def emit_phase_c(b, a, attn_sb):
                for nt in range(4):
                    for jb in range(4):
                        # alternate pools: ps_av is idle during phase C, so
                        # four banks rotate and the PE never waits on copies
                        pool = ps_main if (nt * 4 + jb) % 2 == 0 else ps_av
                        yp = pool.tile([128, 512], F32, tag="mm" if pool is ps_main else "av", name="yp")
                        for mh in range(HPC):
                            nc.tensor.matmul(
                                yp[:],
                                attn_sb[:, mh, nt * 128 : (nt + 1) * 128],
                                wo_sb[:, mh, jb * 512 : (jb + 1) * 512],
                                start=(mh == 0),
                                stop=(mh == HPC - 1),
                            )
                        yt = ystg.tile([128, 512], dt_mm, tag="y", name="yt")
                        if (nt * 4 + jb) % 2 == 0:
                            nc.scalar.copy(yt[:], yp[:])
                        else:
                            nc.vector.tensor_copy(yt[:], yp[:])
                        rt = (b * T + a * 512 + nt * 128) // 128
                        if (nt * 4 + jb) % 2 == 0:
                            nc.gpsimd.dma_start(y[rt, jb], yt[:])
                        else:
                            nc.sync.dma_start(y[rt, jb], yt[:])

            pending_c = None

            for _rep in range(nrep):
              for b in range(B):
                  # k/v for the whole sequence of this batch accumulate here
                  k_sb = kvpool.tile([D, HPC, T], dt_mm, tag="k")
                  v_sb = kvpool.tile([128, HPC, T // 128, D], BF16, tag="v")

                  for a in range(4):  # 512-token block
                      t0 = a * 512
                      q_sb = qpool.tile([D, HPC, 512], dt_mm, tag="q")
                      attn_sb = attnpool.tile([D, HPC, 512], dt_mm, tag="attn")

                      # -------- phase A: qkv + rope for tokens [t0, t0+512)
                      x_t = xpool.tile([128, KT, 512], dt_mm, tag="x")
                      if b == 0 and a == 0:
                          for xc in range(8):
                              nc.sync.dma_start(
                                  x_t[:, 2 * xc : 2 * xc + 2, :],
                                  xT[0, :, 2 * xc : 2 * xc + 2, :],
                              )
                      else:
                          nc.sync.dma_start(x_t[:], xT[b * 4 + a])
                      cos_t = csp.tile([D, 512], F32, tag="cos")
                      sin_t = csp.tile([D, 512], F32, tag="sin")
                      nc.sync.dma_start(cos_t[:], cosT[:, t0 : t0 + 512])
                      nc.sync.dma_start(sin_t[:], sinT[:, t0 : t0 + 512])

                      for m in range(4):  # q0 q1 k0 k1
                          h = m % 2
                          ps = ps_main.tile([128, 512], F32, tag="mm", name="mm")
                          w_sb = wqk[m]
                          for kt in range(KT):
                              nc.tensor.matmul(
                                  ps[:],
                                  w_sb[:, kt, h * D : (h + 1) * D],
                                  x_t[:, kt, :],
                                  start=(kt == 0),
                                  stop=(kt == KT - 1),
                              )
                          # rope: dst = ps*cos + rot(ps)*sin, 4 DVE ops
                          t1 = tmpp.tile([128, 512], F32, tag="tmp", name="t1")
                          rot = tmpp.tile([128, 512], F32, tag="tmp", name="rot")
                          nc.vector.tensor_tensor(t1[:], ps[:], cos_t[:], ALU.mult)
                          nc.vector.scalar_tensor_tensor(
                              rot[0:64, :], ps[64:128, :], -1.0, sin_t[0:64, :],
                              ALU.mult, ALU.mult,
                          )
                          nc.vector.tensor_tensor(
                              rot[64:128, :], ps[0:64, :], sin_t[64:128, :], ALU.mult
                          )
                          dst = q_sb if m < 2 else k_sb
                          col = 0 if m < 2 else t0
                          nc.vector.tensor_tensor(
                              dst[:, h, col : col + 512], t1[:], rot[:], ALU.add
                          )

                      for nt in range(4):  # v in [t, d] layout directly
                          vp_full = ps_main.tile([128, 512], F32, tag="mm", name="vp")
                          vp = vp_full[:, :M]
                          for kt in range(KT):
                              nc.tensor.matmul(
                                  vp,
                                  x_t[:, kt, nt * 128 : (nt + 1) * 128],
                                  wv_sb[:, kt, :],
                                  start=(kt == 0),
                                  stop=(kt == KT - 1),
                              )
                          jt = a * 4 + nt
                          for h in range(HPC):
                              nc.scalar.copy(
                                  v_sb[:, h, jt, :], vp[:, h * D : (h + 1) * D]
                              )

                      # -------- deferred phase C of the previous block
                      if pending_c is not None:
                          emit_phase_c(*pending_c)
                          pending_c = None

                      # -------- phase B: attention for i-block a, both heads,
                      # software-pipelined: S/exp for jt, then AV for jt-1
                      njt = 4 * a + 4
                      avs = [
                          ps_av.tile([128, 512], F32, tag="av", name=f"av{_h}")
                          for _h in range(HPC)
                      ]
                      ptsum2 = ptsump.tile([128, 1024], F32R, tag="ptsum", name="pts")
                      prev = None
                      for jt in range(njt + 1):
                          if jt < njt:
                              diag = jt >= 4 * a
                              o = jt * 128 - a * 512 if diag else 0
                              ob = o if (diag and TRIM_DIAG) else 0
                              sp2 = ps_misc.tile([128, 1024], F32, tag="misc", name="sp2")
                              for h in range(HPC):
                                  if diag:
                                      nc.tensor.matmul(
                                          sp2[:, h * 512 : (h + 1) * 512],
                                          ident_sb[:],
                                          mask_sb[:, 384 - o : 896 - o],
                                          start=True,
                                          stop=False,
                                          skip_group_check=True,
                                      )
                                  nc.tensor.matmul(
                                      sp2[:, h * 512 + ob : (h + 1) * 512],
                                      k_sb[:, h, jt * 128 : (jt + 1) * 128],
                                      q_sb[:, h, ob:512],
                                      start=not diag,
                                      stop=True,
                                      skip_group_check=True,
                                  )
                              # bf16: scores reach ~exp(21) here (randn freqs
                              # make rope a random scaling), far past fp16 range
                              pt2 = ptpool.tile([128, 1024], BF16, tag="pt", name="pt2")
                              nc.scalar.activation(pt2[:], sp2[:], AF.Exp, scale=SCALE)
                              if diag and o > 0:
                                  for h in range(HPC):
                                      nc.gpsimd.memset(
                                          pt2[:, h * 512 : h * 512 + o], 0.0
                                      )
                              cur = (jt, ob, pt2)
                          else:
                              cur = None
                          if prev is not None:
                              pjt, pob, ppt = prev
                              if pjt == 0:
                                  nc.vector.tensor_copy(ptsum2[:], ppt[:])
                              else:
                                  nc.vector.tensor_tensor(
                                      ptsum2[:], ptsum2[:], ppt[:], ALU.add
                                  )
                              for h in range(HPC):
                                  nc.tensor.matmul(
                                      avs[h][:, pob:512],
                                      v_sb[:, h, pjt, :],
                                      ppt[:, h * 512 + pob : (h + 1) * 512],
                                      start=(pjt == 0),
                                      stop=(pjt == njt - 1),
                                      skip_group_check=True,
                                  )
                          prev = cur

                      rsp2 = ps_misc.tile([128, 1024], F32, tag="misc", name="rsp2")
                      for h in range(HPC):
                          nc.tensor.matmul(
                              rsp2[:, h * 512 : (h + 1) * 512],
                              ones_sb[:],
                              ptsum2[:, h * 512 : (h + 1) * 512],
                              start=True,
                              stop=True,
                          )
                      # 1/rs = exp(-ln(rs)) on ACT (DVE has no divide, and the
                      # custom-op fast reciprocal fails this walrus codegen)
                      ln_rs = recp.tile([128, 1024], F32, tag="rec", name="ln_rs")
                      nc.scalar.activation(ln_rs[:], rsp2[:], AF.Ln)
                      rec2 = recp.tile([128, 1024], F32, tag="rec", name="rec2")
                      nc.scalar.activation(rec2[:], ln_rs[:], AF.Exp, scale=-1.0)
                      for h in range(HPC):
                          nc.vector.tensor_tensor(
                              attn_sb[:, h, :],
                              avs[h][:],
                              rec2[:, h * 512 : (h + 1) * 512],
                              ALU.mult,
                          )

                      pending_c = (b, a, attn_sb)

              if pending_c is not None:
                  emit_phase_c(*pending_c)
                  pending_c = None
    return nc


_NC_CACHE = {}


def _get_nc(dt_mm=None, **kw):
    if dt_mm is None:
        dt_mm = DT_MM
    key = (str(dt_mm), tuple(sorted(kw.items())))
    if key not in _NC_CACHE:
        _NC_CACHE[key] = build_kernel(dt_mm, **kw)
    return _NC_CACHE[key]


def make_inputs(x, freqs_cos, freqs_sin, wq, wk, wv, wo):
    """Host-side shard prep: returns in_maps for the 8 cores."""
    x = np.asarray(x, dtype=np.float32)
    # blocked xT: [BT/512 blocks, 128 p, KT, 512 tokens], contiguous per block
    xT = np.ascontiguousarray(
        x.reshape(BT // 512, 512, KT, 128).transpose(0, 3, 2, 1)
    ).astype(np.float16)
    cosT = np.ascontiguousarray(np.asarray(freqs_cos, np.float32).T)
    sinT = np.ascontiguousarray(np.asarray(freqs_sin, np.float32).T)
    p = np.arange(128)[:, None]
    g = np.arange(896)[None, :]
    # additive pre-scale mask: 0 where j<=i (valid), NEG where masked
    maskA = np.where(p <= g - 384, 0.0, NEG).astype(np.float16)
    ones = np.ones((128, 128), np.float32)
    ident = np.eye(128, dtype=np.float16)
    in_maps = []
    for c in range(N_CORES):
        sl = slice(c * M, (c + 1) * M)
        in_maps.append(
            {
                "xT": xT,
                "wqT": np.ascontiguousarray(np.asarray(wq, np.float32)[sl, :].T).astype(np.float16),
                "wkT": np.ascontiguousarray(np.asarray(wk, np.float32)[sl, :].T).astype(np.float16),
                "wvT": np.ascontiguousarray(np.asarray(wv, np.float32)[sl, :].T).astype(np.float16),
                "woT": np.ascontiguousarray(np.asarray(wo, np.float32)[:, sl].T).astype(np.float16),
                "cosT": cosT,
                "sinT": sinT,
                "maskA": maskA,
                "ones": ones,
                "ident": ident,
            }
        )
    return in_maps


def kernel(x, freqs_cos, freqs_sin, wq, wk, wv, wo):
    nc = _get_nc()
    in_maps = make_inputs(x, freqs_cos, freqs_sin, wq, wk, wv, wo)
    res = run_bass_kernel_spmd(nc, in_maps, list(range(N_CORES)))
    out = np.zeros((BT // 128, C // 512, 128, 512), np.float32)
    for r in res.results:
        out += r["y"].astype(np.float32)
    # un-block: [BT/128, C/512, 128, 512] -> [BT, C]
    return out.transpose(0, 2, 1, 3).reshape(B, T, C)


# revision 22
# speedup vs baseline: 1.0491x; 1.0491x over previous
"""Causal self-attention (B=4, T=2048, C=2048, H=16, rope) on 8 trn2 cores.

Sharding: tensor-parallel over heads — 2 heads per core. Each core computes
q/k/v projections for its head slice from the full x, runs causal attention,
and produces a partial output projection y_c = attn_c @ wo[:, slice].T.
The host sums the 8 partial y tensors (row-parallel linear unshard).

v2 layout (fp16 matmuls; "T" suffix = contraction dim on SBUF partitions):
  qT/kT [d=128, t]  <- wT (stationary) x xT (moving) matmuls + rope on DVE
  v     [t=128, d]  <- xT-tile (stationary) x wvT (moving) matmuls, ACT copy
  S^T   [j, 2*i]    <- head-paired [128,1024] PSUM tile; causal mask written
                       by an identity matmul (start=True), S accumulates onto
                       it (start=False) with diagonal tiles column-trimmed
  P^T = exp(S^T * scale) on ACT (one op per jt for both heads); row-sum
                       accumulated on DVE (fp16); one ones-matmul per head
                       broadcasts it; reciprocal_approx_fast + multiply on DVE
  attn^T [d, i]     <- v-tile (stationary) x P^T-half (moving), PSUM-accum
  y[n, j]           <- attnT-tile (stationary) x woT (moving); PSUM->SBUF
                       copies split between ACT and DVE

Scheduling: the attention jt-loop is software-pipelined (S/exp for jt issue
before AV for jt-1) so the exp latency hides under PE work, and phase C of
block a is emitted after phase A of block a+1 so the softmax tail latency
hides under the next block's projection matmuls.
"""

import numpy as np

import concourse.bass as bass
import concourse.mybir as mybir
import concourse.tile as tile
from concourse.vector_clock import ScopedClock
from concourse.bass_utils import run_bass_kernel_spmd

# ---------------------------------------------------------------- tile patch
# The pinned walrus codegen accepts at most ONE sync-wait per hardware
# instruction; Tile attaches several. Split extras onto same-engine NOPs.

_MAX_WAITS = 1
_orig_add_instruction = tile.TileContext._add_instruction


def _split_add_instruction(self, inst):
    si = getattr(inst, "sync_info", None)
    if si is not None and len(si.on_wait) > _MAX_WAITS:
        waits = list(si.on_wait)
        extras, keep = waits[:-_MAX_WAITS], waits[-_MAX_WAITS:]
        inst.sync_info = mybir.SyncInfo(on_wait=keep, on_update=list(si.on_update))
        for i in range(0, len(extras), _MAX_WAITS):
            nop = mybir.InstNoOp(
                name=f"{inst.name}-ws{i}",
                sync_info=mybir.SyncInfo(on_wait=extras[i : i + _MAX_WAITS], on_update=[]),
                engine=inst.engine,
                bass_nofuse=True,
            )
            _orig_add_instruction(self, nop)
    _orig_add_instruction(self, inst)


def _patched_drain_and_barrier(self, tick_clock, wait_clock):
    nc = self.nc
    drain_inst = nc.sync.drain()
    wait_clock.add_sem_waits(drain_inst.ins, ScopedClock({None: tick_clock.global_clock}))
    si = drain_inst.ins.sync_info
    if si is not None and len(si.on_wait) > 1:
        waits = list(si.on_wait)
        drain_inst.ins.sync_info = mybir.SyncInfo(on_wait=waits[:1], on_update=list(si.on_update))
        for w in waits[1:]:
            extra = nc.sync.drain()
            extra.ins.sync_info = mybir.SyncInfo(on_wait=[w], on_update=[])
    nc.all_engine_barrier()
    assert self.sems is not None
    popped = nc._tile_sem_poison_stack.pop()
    assert popped is self._sem_poison
    nc.clear_and_free_semaphores(list(self.sems.allocated().values()))
    nc.all_engine_barrier()


tile.TileContext._add_instruction = _split_add_instruction
tile.TileContext._drain_and_barrier = _patched_drain_and_barrier

# ---------------------------------------------------------------- constants

B, T, C, H, D = 4, 2048, 2048, 16, 128
N_CORES = 8
HPC = H // N_CORES        # heads per core = 2
M = HPC * D               # per-core projection width = 256
BT = B * T
KT = C // 128             # 16 k-subtiles
SCALE = 1.0 / float(np.sqrt(D))
NEG = -30000.0            # pre-scale additive mask value; exp(scale*(s+NEG)) == 0

F32 = mybir.dt.float32
F32R = mybir.dt.float32r
F16 = mybir.dt.float16
BF16 = mybir.dt.bfloat16

DT_MM = F16
ALU = mybir.AluOpType
AF = mybir.ActivationFunctionType

TRIM_DIAG = True          # trim diagonal S matmul moving to [o:512]


def build_kernel(dt_mm=DT_MM, nrep=1):
    nc = bass.Bass("TRN2", target_bir_lowering=False, debug=False)

    xT = nc.dram_tensor("xT", [BT // 512, 128, KT, 512], dt_mm, kind="ExternalInput").ap()
    wqT = nc.dram_tensor("wqT", [C, M], dt_mm, kind="ExternalInput").ap()
    wkT = nc.dram_tensor("wkT", [C, M], dt_mm, kind="ExternalInput").ap()
    wvT = nc.dram_tensor("wvT", [C, M], dt_mm, kind="ExternalInput").ap()
    woT = nc.dram_tensor("woT", [M, C], dt_mm, kind="ExternalInput").ap()
    cosT = nc.dram_tensor("cosT", [D, T], F32, kind="ExternalInput").ap()
    sinT = nc.dram_tensor("sinT", [D, T], F32, kind="ExternalInput").ap()
    maskA = nc.dram_tensor("maskA", [128, 896], dt_mm, kind="ExternalInput").ap()
    ones = nc.dram_tensor("ones", [128, 128], F32R, kind="ExternalInput").ap()
    ident = nc.dram_tensor("ident", [128, 128], dt_mm, kind="ExternalInput").ap()
    y = nc.dram_tensor("y", [BT // 128, C // 512, 128, 512], dt_mm, kind="ExternalOutput").ap()

    with tile.TileContext(nc) as tc:
        with (
            tc.tile_pool(name="const", bufs=1) as constp,
            tc.tile_pool(name="cs", bufs=2) as csp,
            tc.tile_pool(name="xpool", bufs=3) as xpool,
            tc.tile_pool(name="qpool", bufs=2) as qpool,
            tc.tile_pool(name="kvpool", bufs=2) as kvpool,
            tc.tile_pool(name="attnpool", bufs=2) as attnpool,
            tc.tile_pool(name="ptpool", bufs=3) as ptpool,
            tc.tile_pool(name="ptsump", bufs=2) as ptsump,
            tc.tile_pool(name="tmp", bufs=4) as tmpp,
            tc.tile_pool(name="recp", bufs=2) as recp,
            tc.tile_pool(name="ystg", bufs=10) as ystg,
            tc.tile_pool(name="ps_main", bufs=2, space="PSUM") as ps_main,
            tc.tile_pool(name="ps_misc", bufs=2, space="PSUM") as ps_misc,
            tc.tile_pool(name="ps_av", bufs=2, space="PSUM") as ps_av,
        ):
            # ---- resident constants; wq first so phase A can start early
            wq_sb = constp.tile([128, KT, M], dt_mm, tag="wq")
            wk_sb = constp.tile([128, KT, M], dt_mm, tag="wk")
            wv_sb = constp.tile([128, KT, M], dt_mm, tag="wv")
            wo_sb = constp.tile([128, HPC, C], dt_mm, tag="wo")
            mask_sb = constp.tile([128, 896], dt_mm, tag="mask")
            ones_sb = constp.tile([128, 128], F32R, tag="ones")
            ident_sb = constp.tile([128, 128], dt_mm, tag="ident")
            wqR = wqT.rearrange("(ko p) m -> p ko m", p=128)
            for wc in range(4):  # chunked so the first matmuls start early
                nc.gpsimd.dma_start(wq_sb[:, 4 * wc : 4 * wc + 4, :], wqR[:, 4 * wc : 4 * wc + 4, :])
            nc.gpsimd.dma_start(wk_sb[:], wkT.rearrange("(ko p) m -> p ko m", p=128))
            nc.gpsimd.dma_start(wv_sb[:], wvT.rearrange("(ko p) m -> p ko m", p=128))
            nc.scalar.dma_start(mask_sb[:], maskA[:])
            nc.scalar.dma_start(ones_sb[:], ones[:])
            nc.scalar.dma_start(ident_sb[:], ident[:])
            nc.gpsimd.dma_start(wo_sb[:], woT.rearrange("(mh p) j -> p mh j", p=128))

            wqk = [wq_sb, wq_sb, wk_sb, wk_sb]

            def emit_phase_c(b, a, attn_sb):
                for nt in range(4):
                    for jb in range(4):
                        yp = ps_main.tile([128, 512], F32, tag="mm", name="yp")
                        for mh in range(HPC):
                            nc.tensor.matmul(
                                yp[:],
                                attn_sb[:, mh, nt * 128 : (nt + 1) * 128],
                                wo_sb[:, mh, jb * 512 : (jb + 1) * 512],
                                start=(mh == 0),
                                stop=(mh == HPC - 1),
                            )
                        yt = ystg.tile([128, 512], dt_mm, tag="y", name="yt")
                        if (nt * 4 + jb) % 2 == 0:
                            nc.scalar.copy(yt[:], yp[:])
                        else:
                            nc.vector.tensor_copy(yt[:], yp[:])
                        rt = (b * T + a * 512 + nt * 128) // 128
                        if (nt * 4 + jb) % 2 == 0:
                            nc.gpsimd.dma_start(y[rt, jb], yt[:])
                        else:
                            nc.sync.dma_start(y[rt, jb], yt[:])

            pending_c = None

            for _rep in range(nrep):
              for b in range(B):
                  # k/v for the whole sequence of this batch accumulate here
                  k_sb = kvpool.tile([D, HPC, T], dt_mm, tag="k")
                  v_sb = kvpool.tile([128, HPC, T // 128, D], BF16, tag="v")

                  for a in range(4):  # 512-token block
                      t0 = a * 512
                      q_sb = qpool.tile([D, HPC, 512], dt_mm, tag="q")
                      attn_sb = attnpool.tile([D, HPC, 512], dt_mm, tag="attn")

                      # -------- phase A: qkv + rope for tokens [t0, t0+512)
                      x_t = xpool.tile([128, KT, 512], dt_mm, tag="x")
                      if b == 0 and a == 0:
                          for xc in range(8):
                              nc.sync.dma_start(
                                  x_t[:, 2 * xc : 2 * xc + 2, :],
                                  xT[0, :, 2 * xc : 2 * xc + 2, :],
                              )
                      else:
                          nc.sync.dma_start(x_t[:], xT[b * 4 + a])
                      cos_t = csp.tile([D, 512], F32, tag="cos")
                      sin_t = csp.tile([D, 512], F32, tag="sin")
                      nc.sync.dma_start(cos_t[:], cosT[:, t0 : t0 + 512])
                      nc.sync.dma_start(sin_t[:], sinT[:, t0 : t0 + 512])

                      for m in range(4):  # q0 q1 k0 k1
                          h = m % 2
                          ps = ps_main.tile([128, 512], F32, tag="mm", name="mm")
                          w_sb = wqk[m]
                          for kt in range(KT):
                              nc.tensor.matmul(
                                  ps[:],
                                  w_sb[:, kt, h * D : (h + 1) * D],
                                  x_t[:, kt, :],
                                  start=(kt == 0),
                                  stop=(kt == KT - 1),
                              )
                          # rope: dst = ps*cos + rot(ps)*sin, 4 DVE ops
                          t1 = tmpp.tile([128, 512], F32, tag="tmp", name="t1")
                          rot = tmpp.tile([128, 512], F32, tag="tmp", name="rot")
                          nc.vector.tensor_tensor(t1[:], ps[:], cos_t[:], ALU.mult)
                          nc.vector.scalar_tensor_tensor(
                              rot[0:64, :], ps[64:128, :], -1.0, sin_t[0:64, :],
                              ALU.mult, ALU.mult,
                          )
                          nc.vector.tensor_tensor(
                              rot[64:128, :], ps[0:64, :], sin_t[64:128, :], ALU.mult
                          )
                          dst = q_sb if m < 2 else k_sb
                          col = 0 if m < 2 else t0
                          nc.vector.tensor_tensor(
                              dst[:, h, col : col + 512], t1[:], rot[:], ALU.add
                          )

                      for nt in range(4):  # v in [t, d] layout directly
                          vp_full = ps_main.tile([128, 512], F32, tag="mm", name="vp")
                          vp = vp_full[:, :M]
                          for kt in range(KT):
                              nc.tensor.matmul(
                                  vp,
                                  x_t[:, kt, nt * 128 : (nt + 1) * 128],
                                  wv_sb[:, kt, :],
                                  start=(kt == 0),
                                  stop=(kt == KT - 1),
                              )
                          jt = a * 4 + nt
                          for h in range(HPC):
                              nc.scalar.copy(
                                  v_sb[:, h, jt, :], vp[:, h * D : (h + 1) * D]
                              )

                      # -------- deferred phase C of the previous block
                      if pending_c is not None:
                          emit_phase_c(*pending_c)
                          pending_c = None

                      # -------- phase B: attention for i-block a, both heads,
                      # software-pipelined: S/exp for jt, then AV for jt-1
                      njt = 4 * a + 4
                      avs = [
                          ps_av.tile([128, 512], F32, tag="av", name=f"av{_h}")
                          for _h in range(HPC)
                      ]
                      ptsum2 = ptsump.tile([128, 1024], F32R, tag="ptsum", name="pts")
                      prev = None
                      for jt in range(njt + 1):
                          if jt < njt:
                              diag = jt >= 4 * a
                              o = jt * 128 - a * 512 if diag else 0
                              ob = o if (diag and TRIM_DIAG) else 0
                              sp2 = ps_misc.tile([128, 1024], F32, tag="misc", name="sp2")
                              for h in range(HPC):
                                  if diag:
                                      nc.tensor.matmul(
                                          sp2[:, h * 512 : (h + 1) * 512],
                                          ident_sb[:],
                                          mask_sb[:, 384 - o : 896 - o],
                                          start=True,
                                          stop=False,
                                          skip_group_check=True,
                                      )
                                  nc.tensor.matmul(
                                      sp2[:, h * 512 + ob : (h + 1) * 512],
                                      k_sb[:, h, jt * 128 : (jt + 1) * 128],
                                      q_sb[:, h, ob:512],
                                      start=not diag,
                                      stop=True,
                                      skip_group_check=True,
                                  )
                              # bf16: scores reach ~exp(21) here (randn freqs
                              # make rope a random scaling), far past fp16 range
                              pt2 = ptpool.tile([128, 1024], BF16, tag="pt", name="pt2")
                              nc.scalar.activation(pt2[:], sp2[:], AF.Exp, scale=SCALE)
                              if diag and o > 0:
                                  for h in range(HPC):
                                      nc.gpsimd.memset(
                                          pt2[:, h * 512 : h * 512 + o], 0.0
                                      )
                              cur = (jt, ob, pt2)
                          else:
                              cur = None
                          if prev is not None:
                              pjt, pob, ppt = prev
                              if pjt == 0:
                                  nc.vector.tensor_copy(ptsum2[:], ppt[:])
                              else:
                                  nc.vector.tensor_tensor(
                                      ptsum2[:], ptsum2[:], ppt[:], ALU.add
                                  )
                              for h in range(HPC):
                                  nc.tensor.matmul(
                                      avs[h][:, pob:512],
                                      v_sb[:, h, pjt, :],
                                      ppt[:, h * 512 + pob : (h + 1) * 512],
                                      start=(pjt == 0),
                                      stop=(pjt == njt - 1),
                                      skip_group_check=True,
                                  )
                          prev = cur

                      rsp2 = ps_misc.tile([128, 1024], F32, tag="misc", name="rsp2")
                      for h in range(HPC):
                          nc.tensor.matmul(
                              rsp2[:, h * 512 : (h + 1) * 512],
                              ones_sb[:],
                              ptsum2[:, h * 512 : (h + 1) * 512],
                              start=True,
                              stop=True,
                          )
                      # 1/rs = exp(-ln(rs)) on ACT (DVE has no divide, and the
                      # custom-op fast reciprocal fails this walrus codegen)
                      ln_rs = recp.tile([128, 1024], F32, tag="rec", name="ln_rs")
                      nc.scalar.activation(ln_rs[:], rsp2[:], AF.Ln)
                      rec2 = recp.tile([128, 1024], F32, tag="rec", name="rec2")
                      nc.scalar.activation(rec2[:], ln_rs[:], AF.Exp, scale=-1.0)
                      for h in range(HPC):
                          nc.vector.tensor_tensor(
                              attn_sb[:, h, :],
                              avs[h][:],
                              rec2[:, h * 512 : (h + 1) * 512],
                              ALU.mult,
                          )

                      pending_c = (b, a, attn_sb)

              if pending_c is not None:
                  emit_phase_c(*pending_c)
                  pending_c = None
    return nc


_NC_CACHE = {}


def _get_nc(dt_mm=None, **kw):
    if dt_mm is None:
        dt_mm = DT_MM
    key = (str(dt_mm), tuple(sorted(kw.items())))
    if key not in _NC_CACHE:
        _NC_CACHE[key] = build_kernel(dt_mm, **kw)
    return _NC_CACHE[key]


def make_inputs(x, freqs_cos, freqs_sin, wq, wk, wv, wo):
    """Host-side shard prep: returns in_maps for the 8 cores."""
    x = np.asarray(x, dtype=np.float32)
    # blocked xT: [BT/512 blocks, 128 p, KT, 512 tokens], contiguous per block
    xT = np.ascontiguousarray(
        x.reshape(BT // 512, 512, KT, 128).transpose(0, 3, 2, 1)
    ).astype(np.float16)
    cosT = np.ascontiguousarray(np.asarray(freqs_cos, np.float32).T)
    sinT = np.ascontiguousarray(np.asarray(freqs_sin, np.float32).T)
    p = np.arange(128)[:, None]
    g = np.arange(896)[None, :]
    # additive pre-scale mask: 0 where j<=i (valid), NEG where masked
    maskA = np.where(p <= g - 384, 0.0, NEG).astype(np.float16)
    ones = np.ones((128, 128), np.float32)
    ident = np.eye(128, dtype=np.float16)
    in_maps = []
    for c in range(N_CORES):
        sl = slice(c * M, (c + 1) * M)
        in_maps.append(
            {
                "xT": xT,
                "wqT": np.ascontiguousarray(np.asarray(wq, np.float32)[sl, :].T).astype(np.float16),
                "wkT": np.ascontiguousarray(np.asarray(wk, np.float32)[sl, :].T).astype(np.float16),
                "wvT": np.ascontiguousarray(np.asarray(wv, np.float32)[sl, :].T).astype(np.float16),
                "woT": np.ascontiguousarray(np.asarray(wo, np.float32)[:, sl].T).astype(np.float16),
                "cosT": cosT,
                "sinT": sinT,
                "maskA": maskA,
                "ones": ones,
                "ident": ident,
            }
        )
    return in_maps


def kernel(x, freqs_cos, freqs_sin, wq, wk, wv, wo):
    nc = _get_nc()
    in_maps = make_inputs(x, freqs_cos, freqs_sin, wq, wk, wv, wo)
    res = run_bass_kernel_spmd(nc, in_maps, list(range(N_CORES)))
    out = np.zeros((BT // 128, C // 512, 128, 512), np.float32)
    for r in res.results:
        out += r["y"].astype(np.float32)
    # un-block: [BT/128, C/512, 128, 512] -> [BT, C]
    return out.transpose(0, 2, 1, 3).reshape(B, T, C)


# revision 24
# speedup vs baseline: 1.0767x; 1.0263x over previous
"""Causal self-attention (B=4, T=2048, C=2048, H=16, rope) on 8 trn2 cores.

Sharding: tensor-parallel over heads — 2 heads per core. Each core computes
q/k/v projections for its head slice from the full x, runs causal attention,
and produces a partial output projection y_c = attn_c @ wo[:, slice].T.
The host sums the 8 partial y tensors (row-parallel linear unshard).

v2 layout (fp16 matmuls; "T" suffix = contraction dim on SBUF partitions):
  qT/kT [d=128, t]  <- wT (stationary) x xT (moving) matmuls + rope on DVE
  v     [t=128, d]  <- xT-tile (stationary) x wvT (moving) matmuls, ACT copy
  S^T   [j, 2*i]    <- head-paired [128,1024] PSUM tile, column-trimmed on
                       diagonal tiles; the fixed 128-wide triangular mask
                       strip is accumulated by an identity matmul, and the
                       left-of-strip region is zeroed in pt by Pool memsets
  P^T = exp(S^T * scale) on ACT in bf16 (scores reach exp(21): fp16 would
                       overflow); row-sum accumulated on DVE in f32r; one
                       f32r ones-matmul per head broadcasts it; 1/rs =
                       exp(-ln(rs)) on ACT; multiply on DVE
  attn^T [d, i]     <- v-tile (stationary, bf16) x P^T-half (moving),
                       PSUM-accumulated, diagonal tiles column-trimmed
  y[n, j]           <- attnT-tile (stationary) x woT (moving); PSUM->SBUF
                       copies split between ACT and DVE

Scheduling: the attention jt-loop is software-pipelined (S/exp for jt issue
before AV for jt-1) so the exp latency hides under PE work, and phase C of
block a is emitted after phase A of block a+1 so the softmax tail latency
hides under the next block's projection matmuls.
"""

import numpy as np

import concourse.bass as bass
import concourse.mybir as mybir
import concourse.tile as tile
from concourse.vector_clock import ScopedClock
from concourse.bass_utils import run_bass_kernel_spmd

# ---------------------------------------------------------------- tile patch
# The pinned walrus codegen accepts at most ONE sync-wait per hardware
# instruction; Tile attaches several. Split extras onto same-engine NOPs.

_MAX_WAITS = 1
_orig_add_instruction = tile.TileContext._add_instruction


def _split_add_instruction(self, inst):
    si = getattr(inst, "sync_info", None)
    if si is not None and len(si.on_wait) > _MAX_WAITS:
        waits = list(si.on_wait)
        extras, keep = waits[:-_MAX_WAITS], waits[-_MAX_WAITS:]
        inst.sync_info = mybir.SyncInfo(on_wait=keep, on_update=list(si.on_update))
        for i in range(0, len(extras), _MAX_WAITS):
            nop = mybir.InstNoOp(
                name=f"{inst.name}-ws{i}",
                sync_info=mybir.SyncInfo(on_wait=extras[i : i + _MAX_WAITS], on_update=[]),
                engine=inst.engine,
                bass_nofuse=True,
            )
            _orig_add_instruction(self, nop)
    _orig_add_instruction(self, inst)


def _patched_drain_and_barrier(self, tick_clock, wait_clock):
    nc = self.nc
    drain_inst = nc.sync.drain()
    wait_clock.add_sem_waits(drain_inst.ins, ScopedClock({None: tick_clock.global_clock}))
    si = drain_inst.ins.sync_info
    if si is not None and len(si.on_wait) > 1:
        waits = list(si.on_wait)
        drain_inst.ins.sync_info = mybir.SyncInfo(on_wait=waits[:1], on_update=list(si.on_update))
        for w in waits[1:]:
            extra = nc.sync.drain()
            extra.ins.sync_info = mybir.SyncInfo(on_wait=[w], on_update=[])
    nc.all_engine_barrier()
    assert self.sems is not None
    popped = nc._tile_sem_poison_stack.pop()
    assert popped is self._sem_poison
    nc.clear_and_free_semaphores(list(self.sems.allocated().values()))
    nc.all_engine_barrier()


tile.TileContext._add_instruction = _split_add_instruction
tile.TileContext._drain_and_barrier = _patched_drain_and_barrier

# ---------------------------------------------------------------- constants

B, T, C, H, D = 4, 2048, 2048, 16, 128
N_CORES = 8
HPC = H // N_CORES        # heads per core = 2
M = HPC * D               # per-core projection width = 256
BT = B * T
KT = C // 128             # 16 k-subtiles
SCALE = 1.0 / float(np.sqrt(D))
NEG = -30000.0            # pre-scale additive mask value; exp(scale*(s+NEG)) == 0

F32 = mybir.dt.float32
F32R = mybir.dt.float32r
F16 = mybir.dt.float16
BF16 = mybir.dt.bfloat16

DT_MM = F16
ALU = mybir.AluOpType
AF = mybir.ActivationFunctionType

TRIM_DIAG = True          # trim diagonal S matmul moving to [o:512]


def build_kernel(dt_mm=DT_MM, nrep=1):
    nc = bass.Bass("TRN2", target_bir_lowering=False, debug=False)

    xT = nc.dram_tensor("xT", [BT // 512, 128, KT, 512], dt_mm, kind="ExternalInput").ap()
    wqT = nc.dram_tensor("wqT", [C, M], dt_mm, kind="ExternalInput").ap()
    wkT = nc.dram_tensor("wkT", [C, M], dt_mm, kind="ExternalInput").ap()
    wvT = nc.dram_tensor("wvT", [C, M], dt_mm, kind="ExternalInput").ap()
    woT = nc.dram_tensor("woT", [M, C], dt_mm, kind="ExternalInput").ap()
    cosT = nc.dram_tensor("cosT", [D, T], F32, kind="ExternalInput").ap()
    sinT = nc.dram_tensor("sinT", [D, T], F32, kind="ExternalInput").ap()
    maskA = nc.dram_tensor("maskA", [128, 896], dt_mm, kind="ExternalInput").ap()
    ones = nc.dram_tensor("ones", [128, 128], F32R, kind="ExternalInput").ap()
    ident = nc.dram_tensor("ident", [128, 128], dt_mm, kind="ExternalInput").ap()
    y = nc.dram_tensor("y", [BT // 128, C // 512, 128, 512], dt_mm, kind="ExternalOutput").ap()

    with tile.TileContext(nc) as tc:
        with (
            tc.tile_pool(name="const", bufs=1) as constp,
            tc.tile_pool(name="cs", bufs=2) as csp,
            tc.tile_pool(name="xpool", bufs=3) as xpool,
            tc.tile_pool(name="qpool", bufs=2) as qpool,
            tc.tile_pool(name="kvpool", bufs=2) as kvpool,
            tc.tile_pool(name="attnpool", bufs=2) as attnpool,
            tc.tile_pool(name="ptpool", bufs=4) as ptpool,
            tc.tile_pool(name="ptsump", bufs=2) as ptsump,
            tc.tile_pool(name="tmp", bufs=4) as tmpp,
            tc.tile_pool(name="recp", bufs=2) as recp,
            tc.tile_pool(name="ystg", bufs=10) as ystg,
            tc.tile_pool(name="ps_main", bufs=2, space="PSUM") as ps_main,
            tc.tile_pool(name="ps_misc", bufs=2, space="PSUM") as ps_misc,
            tc.tile_pool(name="ps_av", bufs=2, space="PSUM") as ps_av,
        ):
            # ---- resident constants; wq first so phase A can start early
            wq_sb = constp.tile([128, KT, M], dt_mm, tag="wq")
            wk_sb = constp.tile([128, KT, M], dt_mm, tag="wk")
            wv_sb = constp.tile([128, KT, M], dt_mm, tag="wv")
            wo_sb = constp.tile([128, HPC, C], dt_mm, tag="wo")
            mask_sb = constp.tile([128, 896], dt_mm, tag="mask")
            ones_sb = constp.tile([128, 128], F32R, tag="ones")
            ident_sb = constp.tile([128, 128], dt_mm, tag="ident")
            wqR = wqT.rearrange("(ko p) m -> p ko m", p=128)
            for wc in range(4):  # chunked so the first matmuls start early
                nc.gpsimd.dma_start(wq_sb[:, 4 * wc : 4 * wc + 4, :], wqR[:, 4 * wc : 4 * wc + 4, :])
            nc.gpsimd.dma_start(wk_sb[:], wkT.rearrange("(ko p) m -> p ko m", p=128))
            nc.gpsimd.dma_start(wv_sb[:], wvT.rearrange("(ko p) m -> p ko m", p=128))
            nc.scalar.dma_start(mask_sb[:], maskA[:])
            nc.scalar.dma_start(ones_sb[:], ones[:])
            nc.scalar.dma_start(ident_sb[:], ident[:])
            nc.gpsimd.dma_start(wo_sb[:], woT.rearrange("(mh p) j -> p mh j", p=128))

            wqk = [wq_sb, wq_sb, wk_sb, wk_sb]

            def emit_phase_c(b, a, attn_sb):
                for nt in range(4):
                    for jb in range(4):
                        yp = ps_main.tile([128, 512], F32, tag="mm", name="yp")
                        for mh in range(HPC):
                            nc.tensor.matmul(
                                yp[:],
                                attn_sb[:, mh, nt * 128 : (nt + 1) * 128],
                                wo_sb[:, mh, jb * 512 : (jb + 1) * 512],
                                start=(mh == 0),
                                stop=(mh == HPC - 1),
                            )
                        yt = ystg.tile([128, 512], dt_mm, tag="y", name="yt")
                        if (nt * 4 + jb) % 2 == 0:
                            nc.scalar.copy(yt[:], yp[:])
                        else:
                            nc.vector.tensor_copy(yt[:], yp[:])
                        rt = (b * T + a * 512 + nt * 128) // 128
                        if (nt * 4 + jb) % 2 == 0:
                            nc.gpsimd.dma_start(y[rt, jb], yt[:])
                        else:
                            nc.sync.dma_start(y[rt, jb], yt[:])

            pending_c = None

            for _rep in range(nrep):
              for b in range(B):
                  # k/v for the whole sequence of this batch accumulate here
                  k_sb = kvpool.tile([D, HPC, T], dt_mm, tag="k")
                  v_sb = kvpool.tile([128, HPC, T // 128, D], BF16, tag="v")

                  for a in range(4):  # 512-token block
                      t0 = a * 512
                      q_sb = qpool.tile([D, HPC, 512], dt_mm, tag="q")
                      attn_sb = attnpool.tile([D, HPC, 512], dt_mm, tag="attn")

                      # -------- phase A: qkv + rope for tokens [t0, t0+512)
                      x_t = xpool.tile([128, KT, 512], dt_mm, tag="x")
                      if b == 0 and a == 0:
                          for xc in range(8):
                              nc.sync.dma_start(
                                  x_t[:, 2 * xc : 2 * xc + 2, :],
                                  xT[0, :, 2 * xc : 2 * xc + 2, :],
                              )
                      else:
                          nc.sync.dma_start(x_t[:], xT[b * 4 + a])
                      cos_t = csp.tile([D, 512], F32, tag="cos")
                      sin_t = csp.tile([D, 512], F32, tag="sin")
                      nc.sync.dma_start(cos_t[:], cosT[:, t0 : t0 + 512])
                      nc.sync.dma_start(sin_t[:], sinT[:, t0 : t0 + 512])

                      for m in range(4):  # q0 q1 k0 k1
                          h = m % 2
                          ps = ps_main.tile([128, 512], F32, tag="mm", name="mm")
                          w_sb = wqk[m]
                          for kt in range(KT):
                              nc.tensor.matmul(
                                  ps[:],
                                  w_sb[:, kt, h * D : (h + 1) * D],
                                  x_t[:, kt, :],
                                  start=(kt == 0),
                                  stop=(kt == KT - 1),
                              )
                          # rope: dst = ps*cos + rot(ps)*sin, 4 DVE ops
                          t1 = tmpp.tile([128, 512], F32, tag="tmp", name="t1")
                          rot = tmpp.tile([128, 512], F32, tag="tmp", name="rot")
                          nc.vector.tensor_tensor(t1[:], ps[:], cos_t[:], ALU.mult)
                          nc.vector.scalar_tensor_tensor(
                              rot[0:64, :], ps[64:128, :], -1.0, sin_t[0:64, :],
                              ALU.mult, ALU.mult,
                          )
                          nc.vector.tensor_tensor(
                              rot[64:128, :], ps[0:64, :], sin_t[64:128, :], ALU.mult
                          )
                          dst = q_sb if m < 2 else k_sb
                          col = 0 if m < 2 else t0
                          nc.vector.tensor_tensor(
                              dst[:, h, col : col + 512], t1[:], rot[:], ALU.add
                          )

                      for nt in range(4):  # v in [t, d] layout directly
                          vp_full = ps_main.tile([128, 512], F32, tag="mm", name="vp")
                          vp = vp_full[:, :M]
                          for kt in range(KT):
                              nc.tensor.matmul(
                                  vp,
                                  x_t[:, kt, nt * 128 : (nt + 1) * 128],
                                  wv_sb[:, kt, :],
                                  start=(kt == 0),
                                  stop=(kt == KT - 1),
                              )
                          jt = a * 4 + nt
                          for h in range(HPC):
                              nc.scalar.copy(
                                  v_sb[:, h, jt, :], vp[:, h * D : (h + 1) * D]
                              )

                      # -------- deferred phase C of the previous block
                      if pending_c is not None:
                          emit_phase_c(*pending_c)
                          pending_c = None

                      # -------- phase B: attention for i-block a, both heads,
                      # software-pipelined: S/exp for jt, then AV for jt-1
                      njt = 4 * a + 4
                      avs = [
                          ps_av.tile([128, 512], F32, tag="av", name=f"av{_h}")
                          for _h in range(HPC)
                      ]
                      ptsum2 = ptsump.tile([128, 1024], F32R, tag="ptsum", name="pts")
                      prev = None
                      for jt in range(njt + 1):
                          if jt < njt:
                              diag = jt >= 4 * a
                              o = jt * 128 - a * 512 if diag else 0
                              ob = o if (diag and TRIM_DIAG) else 0
                              sp2 = ps_misc.tile([128, 1024], F32, tag="misc", name="sp2")
                              for h in range(HPC):
                                  if diag:
                                      nc.tensor.matmul(
                                          sp2[:, h * 512 : (h + 1) * 512],
                                          ident_sb[:],
                                          mask_sb[:, 384 - o : 896 - o],
                                          start=True,
                                          stop=False,
                                          skip_group_check=True,
                                      )
                                  nc.tensor.matmul(
                                      sp2[:, h * 512 + ob : (h + 1) * 512],
                                      k_sb[:, h, jt * 128 : (jt + 1) * 128],
                                      q_sb[:, h, ob:512],
                                      start=not diag,
                                      stop=True,
                                      skip_group_check=True,
                                  )
                              # bf16: scores reach ~exp(21) here (randn freqs
                              # make rope a random scaling), far past fp16 range
                              pt2 = ptpool.tile([128, 1024], BF16, tag="pt", name="pt2")
                              nc.scalar.activation(pt2[:], sp2[:], AF.Exp, scale=SCALE)
                              if diag and o > 0:
                                  for h in range(HPC):
                                      nc.gpsimd.memset(
                                          pt2[:, h * 512 : h * 512 + o], 0.0
                                      )
                              cur = (jt, ob, pt2)
                          else:
                              cur = None
                          if prev is not None:
                              pjt, pob, ppt = prev
                              if pjt == 0:
                                  nc.vector.tensor_copy(ptsum2[:], ppt[:])
                              else:
                                  nc.vector.tensor_tensor(
                                      ptsum2[:], ptsum2[:], ppt[:], ALU.add
                                  )
                              for h in range(HPC):
                                  nc.tensor.matmul(
                                      avs[h][:, pob:512],
                                      v_sb[:, h, pjt, :],
                                      ppt[:, h * 512 + pob : (h + 1) * 512],
                                      start=(pjt == 0),
                                      stop=(pjt == njt - 1),
                                      skip_group_check=True,
                                  )
                          prev = cur

                      rsp2 = ps_misc.tile([128, 1024], F32, tag="misc", name="rsp2")
                      for h in range(HPC):
                          nc.tensor.matmul(
                              rsp2[:, h * 512 : (h + 1) * 512],
                              ones_sb[:],
                              ptsum2[:, h * 512 : (h + 1) * 512],
                              start=True,
                              stop=True,
                          )
                      # 1/rs = exp(-ln(rs)) on ACT (DVE has no divide, and the
                      # custom-op fast reciprocal fails this walrus codegen)
                      ln_rs = recp.tile([128, 1024], F32, tag="rec", name="ln_rs")
                      nc.scalar.activation(ln_rs[:], rsp2[:], AF.Ln)
                      rec2 = recp.tile([128, 1024], F32, tag="rec", name="rec2")
                      nc.scalar.activation(rec2[:], ln_rs[:], AF.Exp, scale=-1.0)
                      for h in range(HPC):
                          nc.vector.tensor_tensor(
                              attn_sb[:, h, :],
                              avs[h][:],
                              rec2[:, h * 512 : (h + 1) * 512],
                              ALU.mult,
                          )

                      pending_c = (b, a, attn_sb)

              if pending_c is not None:
                  emit_phase_c(*pending_c)
                  pending_c = None
    return nc


_NC_CACHE = {}


def _get_nc(dt_mm=None, **kw):
    if dt_mm is None:
        dt_mm = DT_MM
    key = (str(dt_mm), tuple(sorted(kw.items())))
    if key not in _NC_CACHE:
        _NC_CACHE[key] = build_kernel(dt_mm, **kw)
    return _NC_CACHE[key]


def make_inputs(x, freqs_cos, freqs_sin, wq, wk, wv, wo):
    """Host-side shard prep: returns in_maps for the 8 cores."""
    x = np.asarray(x, dtype=np.float32)
    # blocked xT: [BT/512 blocks, 128 p, KT, 512 tokens], contiguous per block
    xT = np.ascontiguousarray(
        x.reshape(BT // 512, 512, KT, 128).transpose(0, 3, 2, 1)
    ).astype(np.float16)
    cosT = np.ascontiguousarray(np.asarray(freqs_cos, np.float32).T)
    sinT = np.ascontiguousarray(np.asarray(freqs_sin, np.float32).T)
    p = np.arange(128)[:, None]
    g = np.arange(896)[None, :]
    # additive pre-scale mask: 0 where j<=i (valid), NEG where masked
    maskA = np.where(p <= g - 384, 0.0, NEG).astype(np.float16)
    ones = np.ones((128, 128), np.float32)
    ident = np.eye(128, dtype=np.float16)
    in_maps = []
    for c in range(N_CORES):
        sl = slice(c * M, (c + 1) * M)
        in_maps.append(
            {
                "xT": xT,
                "wqT": np.ascontiguousarray(np.asarray(wq, np.float32)[sl, :].T).astype(np.float16),
                "wkT": np.ascontiguousarray(np.asarray(wk, np.float32)[sl, :].T).astype(np.float16),
                "wvT": np.ascontiguousarray(np.asarray(wv, np.float32)[sl, :].T).astype(np.float16),
                "woT": np.ascontiguousarray(np.asarray(wo, np.float32)[:, sl].T).astype(np.float16),
                "cosT": cosT,
                "sinT": sinT,
                "maskA": maskA,
                "ones": ones,
                "ident": ident,
            }
        )
    return in_maps


def kernel(x, freqs_cos, freqs_sin, wq, wk, wv, wo):
    nc = _get_nc()
    in_maps = make_inputs(x, freqs_cos, freqs_sin, wq, wk, wv, wo)
    res = run_bass_kernel_spmd(nc, in_maps, list(range(N_CORES)))
    out = np.zeros((BT // 128, C // 512, 128, 512), np.float32)
    for r in res.results:
        out += r["y"].astype(np.float32)
    # un-block: [BT/128, C/512, 128, 512] -> [BT, C]
    return out.transpose(0, 2, 1, 3).reshape(B, T, C)
